# revision 8
# baseline (speedup 1.0000x reference)
"""Chamfer loss (two 16384x16384 1-NN searches + gathered MSE) on 8 Trainium2 cores.

Device (per core; queries sharded 8-way across cores, both search directions
per core, 16 query blocks of 128 per direction):
  - One For_i hardware loop, `unroll` blocks per iteration (default 8 -> 2
    iterations/workload), so the NEFF stays small and per-call program-size
    dispatch overhead — which dominated the previous 93.85 ms estimate — is
    gone. `repeat` reruns the whole workload R times in-loop for
    noise-robust timing. Query tiles ping-pong via DMA with loop-var
    (register) DRAM offsets; ref table rt [4, 2V] f16 stays SBUF-resident.
  - Scores: s[i,j] = q_i.r_j - |r_j|^2/2 (query-norm term dropped:
    argmax-invariant) via f16 matmuls [4,128]x[4,512] -> PSUM f32, 16
    half-quads [128,1024] per block-direction (PSUM pool 4 bufs decouples
    PE from evacuation). fp16 input rounding is covered by the top-8 slot
    margin (offline: worst true-slot rank 3 of 1024) + exact host rescore.
  - PSUM evacuation split across engines (walrus: Pool cannot touch PSUM,
    DVE allows only one PSUM operand per op):
      quads 2..7: ScalarE copies psq f32 -> T[(g-2)*2048+...] f16
      quads 0,1:  DVE max(psq half (PSUM), copied quad g+2 half (SBUF))
                  -> T[12288+...], absorbing the partner's values.
  - DVE fold tree to 1024 slots (disjoint dead regions, no in-place):
      t1: T[4096:8192] vs T[8192:12288] -> T[0:4096]
      t2: T[0:4096]    vs T[12288:16384]-> T[4096:8192]
      t3: T[4096:6144] vs T[6144:8192]  -> T[0:2048]
      t4: T[0:1024]    vs T[1024:2048]  -> T[2048:3072]
    Slot v covers candidates j = v + 1024k (k=0..15) — checked symbolically
    in _cover_map(). The tree+top-8 of block-direction k is emitted after
    the evacuation of k+1 (software pipelining), with scratch T per
    direction so consecutive block-directions overlap fully.
  - DVE max/max_index -> top-8 slot ids -> DMA to DRAM at loop-var column.

Host: exact fp32 re-scoring of the 128 candidates per query (same formula
as the reference), first-index argmax -> exact 1-NN index; squared-error
means in f64 -> final f32 scalar.

Engine steady-state per block-direction (CoreSim): DVE ~13.1us (4 drain ops
+ 4 tree ops + max/max_index), Act ~11.9us (12 copies), PE ~7.3us (32
matmuls). DVE-bound; full workload ~485us/core.
"""
import sys

sys.path.insert(0, "/opt/trn_rl_repo")

import numpy as np

import concourse.bass as bass
import concourse.bacc as bacc
import concourse.mybir as mybir
from concourse.bass import ds
from concourse.tile import TileContext
from concourse.bass_utils import run_bass_kernel_spmd

P = 128          # partitions / queries per block
V = 16384        # reference points per direction
NCORES = 8
QPC = V // NCORES            # queries per core per direction (2048)
NBLK = QPC // P              # query blocks per core per direction (16)
NSLOT = 512                  # folded slots per query
NCAND = 256                  # candidates per query: top-8 slots x 32-fold
F16 = mybir.dt.float16
F32 = mybir.dt.float32
U16 = mybir.dt.uint16
MAX = mybir.AluOpType.max

_CACHE = {}


def build(n_blocks=NBLK, repeat=1, unroll=8, staggered=False):
    assert n_blocks % unroll == 0
    nc = bacc.Bacc()
    qT = nc.dram_tensor("qT", [4, 2 * QPC], F16, kind="ExternalInput")
    rT = nc.dram_tensor("rT", [4, 2 * V], F16, kind="ExternalInput")
    slot_out = nc.dram_tensor(
        "slot_out", [P, 2 * n_blocks * 8], U16, kind="ExternalOutput"
    )
    span = n_blocks * P

    with TileContext(nc) as tc:
        with (
            tc.tile_pool(name="tab", bufs=1) as tab,
            tc.tile_pool(name="qb", bufs=1) as qb,
            tc.tile_pool(name="fold", bufs=1) as fold,
            tc.tile_pool(name="small", bufs=1) as sm,
            tc.tile_pool(name="ps", bufs=4, space="PSUM") as ps,
        ):
            rt = tab.tile([4, 2 * V], F16)
            qblk = [
                qb.tile([4, 2 * P], F16, name=f"qblk{u}") for u in range(unroll)
            ]
            T = [
                fold.tile([P, 16384], F16, tag=f"T_{d}", name=f"T_{d}")
                for d in range(2)
            ]
            m8 = [
                sm.tile([P, 8], F16, tag=f"m8_{d}", name=f"m8_{d}")
                for d in range(2)
            ]
            s8 = [
                sm.tile([P, 8], U16, tag=f"s8_{d}", name=f"s8_{d}")
                for d in range(2)
            ]
            for ch in (1, 0, 2, 3, 4, 5, 6, 7):  # first-needed chunk first
                nc.sync.dma_start(
                    out=rt[:, ch * 4096 : (ch + 1) * 4096],
                    in_=rT[:, ch * 4096 : (ch + 1) * 4096],
                )

            def load_qblk(u, q0):
                nc.sync.dma_start(out=qblk[u][:, 0:P], in_=qT[:, ds(q0, P)])
                nc.sync.dma_start(
                    out=qblk[u][:, P : 2 * P], in_=qT[:, ds(QPC + q0, P)]
                )

            def emit_evac(u, d):
                """Matmuls + PSUM evacuation for one block-direction."""
                t = T[d]
                lhsT = qblk[u][:, d * P : (d + 1) * P]
                for g in (2, 3, 0, 1, 4, 5, 6, 7):
                    for h in range(2):
                        psh = ps.tile(
                            [P, 1024], F32, tag="psq", name=f"ps{u}{d}{g}{h}"
                        )
                        for i in range(2):
                            col = (g * 4 + h * 2 + i) * 512
                            nc.tensor.matmul(
                                out=psh[:, i * 512 : (i + 1) * 512],
                                lhsT=lhsT,
                                rhs=rt[:, d * V + col : d * V + col + 512],
                                start=True,
                                stop=True,
                            )
                        if g >= 2:
                            nc.scalar.copy(
                                t[
                                    :,
                                    (g - 2) * 2048
                                    + h * 1024 : (g - 2) * 2048
                                    + (h + 1) * 1024,
                                ],
                                psh[:],
                            )
                        else:
                            nc.vector.tensor_tensor(
                                out=t[
                                    :,
                                    12288
                                    + (2 * g + h) * 1024 : 12288
                                    + (2 * g + h + 1) * 1024,
                                ],
                                in0=psh[:],
                                in1=t[
                                    :,
                                    g * 2048
                                    + h * 1024 : g * 2048
                                    + (h + 1) * 1024,
                                ],
                                op=MAX,
                            )

            def emit_finish(d, q0):
                """Fold tree + top-8 + slot DMA (runs one block-direction late)."""
                t = T[d]
                nc.vector.tensor_tensor(  # t1
                    out=t[:, 0:4096], in0=t[:, 4096:8192], in1=t[:, 8192:12288], op=MAX
                )
                nc.vector.tensor_tensor(  # t2
                    out=t[:, 4096:8192], in0=t[:, 0:4096], in1=t[:, 12288:16384], op=MAX
                )
                nc.vector.tensor_tensor(  # t3
                    out=t[:, 0:2048], in0=t[:, 4096:6144], in1=t[:, 6144:8192], op=MAX
                )
                nc.vector.tensor_tensor(  # t4
                    out=t[:, 2048:3072], in0=t[:, 0:1024], in1=t[:, 1024:2048], op=MAX
                )
                nc.vector.tensor_tensor(  # t5
                    out=t[:, 0:512], in0=t[:, 2048:2560], in1=t[:, 2560:3072], op=MAX
                )
                nc.vector.max(out=m8[d][:], in_=t[:, 0:512])
                nc.vector.max_index(
                    out=s8[d][:], in_max=m8[d][:], in_values=t[:, 0:512]
                )
                nc.sync.dma_start(
                    out=slot_out[:, ds(d * n_blocks * 8 + (q0 >> 4), 8)],
                    in_=s8[d][:],
                )

            load_qblk(0, 0)
            step = unroll * P
            with tc.For_i(0, repeat * span, step, staggered_reset=staggered) as it:
                q0s = [
                    nc.s_assert_within(
                        (it + u * P) % span, 0, span - P,
                        skip_runtime_assert=True,
                    )
                    for u in range(unroll)
                ]
                q0n = nc.s_assert_within(
                    (it + step) % span, 0, span - P, skip_runtime_assert=True
                )
                # software-pipelined: finish(bd k) is emitted after evac(bd k+1)
                # so the fold tree overlaps the next block-direction's copies.
                pending = None
                for u in range(unroll):
                    if u + 1 < unroll:
                        load_qblk(u + 1, q0s[u + 1])
                    else:
                        load_qblk(0, q0n)
                    for d in range(2):
                        emit_evac(u, d)
                        if pending is not None:
                            emit_finish(*pending)
                        pending = (d, q0s[u])
                emit_finish(*pending)
    nc.compile()
    return nc


def _cover_map():
    """Symbolic check: device fold tree slot v covers {v + 1024k}."""
    t = [None] * 16384
    for g in (2, 3, 4, 5, 6, 7):
        for s in range(2048):
            t[(g - 2) * 2048 + s] = {2048 * g + s}
    for g in (0, 1):
        for h in range(2):
            for s in range(1024):
                t[12288 + (2 * g + h) * 1024 + s] = {
                    2048 * g + 1024 * h + s
                } | t[g * 2048 + 1024 * h + s]
    t[0:4096] = [t[4096 + x] | t[8192 + x] for x in range(4096)]
    t[4096:8192] = [t[x] | t[12288 + x] for x in range(4096)]
    t[0:2048] = [t[4096 + x] | t[6144 + x] for x in range(2048)]
    t[2048:3072] = [t[x] | t[1024 + x] for x in range(1024)]
    t[0:512] = [t[2048 + x] | t[2560 + x] for x in range(512)]
    for v in range(512):
        assert t[v] == {v + 512 * k for k in range(32)}, v
    return True


def _aug_tables(pred_vertices, trg_vertices):
    pv = np.ascontiguousarray(pred_vertices[0])  # [V,3]
    tv = np.ascontiguousarray(trg_vertices[0])

    def aug_ref_T(r):  # [4, V]: x, y, z, -|r|^2/2
        n2 = ((r * r).sum(1) * np.float32(0.5)).astype(np.float32)
        return np.concatenate([r.T, -n2[None, :]], axis=0)

    def aug_q_T(q):  # [4, Vq]: x, y, z, 1
        return np.concatenate(
            [q.T, np.ones((1, q.shape[0]), np.float32)], axis=0
        )

    rT = np.ascontiguousarray(
        np.concatenate([aug_ref_T(pv), aug_ref_T(tv)], axis=1).astype(np.float16)
    )
    qT_A = aug_q_T(tv).astype(np.float16)
    qT_B = aug_q_T(pv).astype(np.float16)
    return pv, tv, rT, qT_A, qT_B


def _prep_inputs(pred_vertices, trg_vertices, pred_e=None, trg_e=None):
    _, _, rT, qT_A, qT_B = _aug_tables(pred_vertices, trg_vertices)
    in_maps = []
    for c in range(NCORES):
        sl = slice(c * QPC, (c + 1) * QPC)
        in_maps.append(
            {
                "qT": np.ascontiguousarray(
                    np.concatenate([qT_A[:, sl], qT_B[:, sl]], axis=1)
                ),
                "rT": rT,
            }
        )
    return in_maps


def run_device(in_maps):
    if "nc" not in _CACHE:
        _CACHE["nc"] = build()
    return run_bass_kernel_spmd(_CACHE["nc"], in_maps, list(range(NCORES))).results


def _exact_indices(results, pv, tv):
    """Top-8 slots -> 128 candidates (v + 1024k) -> exact fp32 argmax."""
    out = []
    offs = (np.arange(32, dtype=np.int64) * 512)[None, None, :]
    for d, (q, r) in enumerate([(tv, pv), (pv, tv)]):
        slots = np.empty((V, 8), np.int64)
        for c in range(NCORES):
            so = results[c]["slot_out"]  # [P, 2*NBLK*8]
            for b in range(NBLK):
                rows = slice(c * QPC + b * P, c * QPC + (b + 1) * P)
                slots[rows] = so[:, (d * NBLK + b) * 8 : (d * NBLK + b + 1) * 8]
        cand = (slots[:, :, None] + offs).reshape(V, NCAND)  # [V, 256]
        n2 = ((r * r).sum(1) * np.float32(0.5)).astype(np.float32)
        rc = r[cand]                            # [V, 256, 3]
        s = np.einsum("vkc,vc->vk", rc, q).astype(np.float32) - n2[cand]
        smax = s.max(axis=1)
        masked = np.where(s >= smax[:, None], cand, 1 << 30)
        out.append(masked.min(axis=1))
    return out  # [idxA, idxB]


def kernel(pred_vertices, trg_vertices, pred_e, trg_e):
    pv, tv, _, _, _ = _aug_tables(pred_vertices, trg_vertices)
    in_maps = _prep_inputs(pred_vertices, trg_vertices)
    results = run_device(in_maps)
    idxA, idxB = _exact_indices(results, pv, tv)
    pe = np.ascontiguousarray(pred_e[0])
    te = np.ascontiguousarray(trg_e[0])
    lossA = ((te.astype(np.float64) - pe[idxA].astype(np.float64)) ** 2).sum() / (
        V * 3
    )
    lossB = ((pe.astype(np.float64) - te[idxB].astype(np.float64)) ** 2).sum() / (
        V * 3
    )
    return np.float32(lossA + lossB)


def kernel_indices(pred_vertices, trg_vertices, pred_e=None, trg_e=None):
    pv, tv, _, _, _ = _aug_tables(pred_vertices, trg_vertices)
    in_maps = _prep_inputs(pred_vertices, trg_vertices)
    results = run_device(in_maps)
    return _exact_indices(results, pv, tv)
